# revision 1
# baseline (speedup 1.0000x reference)
import sys
import numpy as np

sys.path.insert(0, "/opt/trn_rl_repo")

import concourse.bass as bass  # noqa: E402
import concourse.bacc as bacc  # noqa: E402
import concourse.tile as tile  # noqa: E402
from concourse import mybir  # noqa: E402
from concourse.bass_utils import run_bass_kernel_spmd  # noqa: E402

# Problem dims (hardcoded per spec)
N, T, V, C_IN, C_OUT, K, KT = 256, 2048, 9, 16, 3, 5, 9
F_IN = V * C_IN    # 144
F_OUT = V * C_OUT  # 27
N_CORES = 8
N_PER_CORE = N // N_CORES  # 32

F32 = mybir.dt.float32
F16 = mybir.dt.float16

_PROGRAM_CACHE = {}


def _build_program():
    nc = bacc.Bacc()

    # poseT: feature-major input, rows 0..127 = x.T rows 0..127 (per sample)
    poseT = nc.declare_dram_parameter("poseT", [N_PER_CORE, 128, T], F16, isOutput=False)
    # pbs4: per 4-sample group, rows 32k+0..16 = feats 128..143 + ones, rest 0
    pbs4 = nc.declare_dram_parameter("pbs4", [N_PER_CORE // 4, 128, T], F16, isOutput=False)
    waT = nc.declare_dram_parameter("waT", [128, 32], F16, isOutput=False)
    wb4 = nc.declare_dram_parameter("wb4", [128, 128], F16, isOutput=False)
    ct = nc.declare_dram_parameter("ct", [KT, 128, 128], F16, isOutput=False)
    bias4 = nc.declare_dram_parameter("bias4", [128, 1], F32, isOutput=False)
    # raw tile dump: [group, chunk, (4 samples x 32ch), 512t]; host unpacks
    out = nc.declare_dram_parameter(
        "out", [N_PER_CORE // 4, T // 512, 128, 512], F16, isOutput=True)

    NG = N_PER_CORE // 4  # groups of 4 samples
    NC_T = T // 512       # 512-col chunks per sample

    with tile.TileContext(nc) as tc:
        with (
            tc.tile_pool(name="const", bufs=1) as cpool,
            tc.tile_pool(name="poseT", bufs=3) as ppool,
            tc.tile_pool(name="zbuf", bufs=3) as zpool,
            tc.tile_pool(name="outsb", bufs=3) as opool,
            tc.tile_pool(name="psZ", bufs=4, space=bass.MemorySpace.PSUM) as psZ_p,
            tc.tile_pool(name="psO", bufs=4, space=bass.MemorySpace.PSUM) as psO_p,
        ):
            # ---- constants ----
            waT_sb = cpool.tile([128, 32], F16, tag="waT")
            wb4_sb = cpool.tile([128, 128], F16, tag="wb4")
            ct_sb = [
                cpool.tile([128, 128], F16, tag=f"ct{i}", name=f"ct_sb{i}")
                for i in range(KT)
            ]
            bias4_sb = cpool.tile([128, 1], F32, tag="bias4")

            nc.scalar.dma_start(waT_sb[:], waT[:])
            nc.scalar.dma_start(wb4_sb[:], wb4[:])
            for i in range(KT):
                nc.scalar.dma_start(ct_sb[i][:], ct[i])
            nc.scalar.dma_start(bias4_sb[:], bias4[:])

            for g in range(NG):
                # ---- load 4 samples (feature-major already) ----
                pta = []
                for kk in range(4):
                    n = 4 * g + kk
                    pa = ppool.tile([128, T], F16, tag=f"pta{kk}", name=f"pa{kk}")
                    nc.sync.dma_start(pa[:], poseT[n, 0:128, :])
                    pta.append(pa)
                pbs = ppool.tile([128, T], F16, tag="pbs", name="pbs")
                nc.sync.dma_start(pbs[:], pbs4[g])

                # ---- GCN: z[128=(4k x 32ch), t] ----
                zb = zpool.tile([128, T + 8], F16, tag="zb")
                nc.vector.memset(zb[:, 0:4], 0.0)
                nc.vector.memset(zb[:, T + 4:T + 8], 0.0)
                for c in range(NC_T):
                    psZ = psZ_p.tile([128, 512], F32, tag="psZ")
                    sl = slice(c * 512, (c + 1) * 512)
                    # A-chunks first (each clears its own col-group) so chunk
                    # start doesn't wait on the stacked-B load; B accumulates
                    # full-width last.
                    for kk in range(4):
                        nc.tensor.matmul(
                            psZ[32 * kk:32 * kk + 32, :], waT_sb[:], pta[kk][:, sl],
                            start=True, stop=False, tile_position=(0, 32 * kk),
                        )
                    nc.tensor.matmul(
                        psZ[:], wb4_sb[:], pbs[:, sl],
                        start=False, stop=True,
                    )
                    nc.vector.tensor_copy(zb[:, 4 + c * 512:4 + (c + 1) * 512], psZ[:])

                # ---- conv + bias + leaky relu + store ----
                for c in range(NC_T):
                    psO = psO_p.tile([128, 512], F32, tag="psO")
                    for it in range(KT):
                        nc.tensor.matmul(
                            psO[:], ct_sb[it][:],
                            zb[:, c * 512 + it:c * 512 + it + 512],
                            start=(it == 0), stop=(it == KT - 1),
                        )
                    osb = opool.tile([128, 512], F16, tag="osb")
                    nc.scalar.activation(
                        osb[:], psO[:], mybir.ActivationFunctionType.Lrelu,
                        bias=bias4_sb[:, 0:1], alpha=0.01,
                    )
                    eng = nc.scalar if c % 2 == 0 else nc.sync
                    eng.dma_start(out[g, c], osb[:])

    nc.finalize()
    return nc


def _host_consts(A, W_gcn, b_gcn, W_tcn, b_tcn):
    A = np.asarray(A, np.float32)
    W_gcn = np.asarray(W_gcn, np.float32)
    b_gcn = np.asarray(b_gcn, np.float32)
    W_tcn = np.asarray(W_tcn, np.float32)
    b_tcn = np.asarray(b_tcn, np.float32)

    # W_eff[(v,c),(w,o)] = sum_k W_gcn[k,o,c] A[k,v,w]
    W_eff = np.einsum("koc,kvw->vcwo", W_gcn, A).reshape(F_IN, F_OUT).astype(np.float32)
    b_eff = np.einsum("ko,kw->wo", b_gcn, A.sum(axis=1)).reshape(F_OUT).astype(np.float32)

    waT = np.zeros((128, 32), np.float32)
    waT[:, :F_OUT] = W_eff[:128]
    wb1 = np.zeros((17, 32), np.float32)
    wb1[:16, :F_OUT] = W_eff[128:]
    wb1[16, :F_OUT] = b_eff  # multiplied by the ones row
    wb4 = np.zeros((128, 128), np.float32)
    for kk in range(4):
        wb4[32 * kk:32 * kk + 17, 32 * kk:32 * kk + 32] = wb1

    # conv taps: C_tau[i,o] = W_tcn[o,i,4-tau]; block-diag over (4 samples x 32) with
    # within-32 block-diag over joints w: (w,i) -> (w,o)
    ct = np.zeros((KT, 128, 128), np.float32)
    for it, tau in enumerate(range(-4, 5)):
        Ct = W_tcn[:, :, 4 - tau, 0].T  # [i, o]
        blk = np.zeros((32, 32), np.float32)
        for w in range(V):
            blk[3 * w:3 * w + 3, 3 * w:3 * w + 3] = Ct
        for kk in range(4):
            ct[it, 32 * kk:32 * kk + 32, 32 * kk:32 * kk + 32] = blk

    bias4 = np.zeros((128, 1), np.float32)
    for kk in range(4):
        bias4[32 * kk:32 * kk + F_OUT, 0] = np.tile(b_tcn, V)

    f16 = np.float16
    return waT.astype(f16), wb4.astype(f16), ct.astype(f16), bias4


def _host_transpose(pose):
    # -> poseT [N, 128, T] (feats 0..127) and pbs4 [N//4, 128, T]
    # (rows 32k+0..15 = feats 128..143 of sample 4g+k, row 32k+16 = ones)
    x16 = np.swapaxes(pose.astype(np.float16), 1, 2)  # [N, 144, T]
    poseT = np.ascontiguousarray(x16[:, :128, :])
    pbs4 = np.zeros((N // 4, 128, T), np.float16)
    for kk in range(4):
        pbs4[:, 32 * kk:32 * kk + 16, :] = x16[kk::4][:, 128:144, :]
        pbs4[:, 32 * kk + 16, :] = np.float16(1.0)
    return poseT, pbs4


def _run(inputs, **spmd_kwargs):
    pose = np.asarray(inputs["pose_feats"], np.float32)
    poseT, pbs4 = _host_transpose(pose)
    waT, wb4, ct, bias4 = _host_consts(
        inputs["A"], inputs["W_gcn"], inputs["b_gcn"], inputs["W_tcn"], inputs["b_tcn"]
    )

    if "prog" not in _PROGRAM_CACHE:
        _PROGRAM_CACHE["prog"] = _build_program()
    nc = _PROGRAM_CACHE["prog"]

    in_maps = []
    for i in range(N_CORES):
        in_maps.append({
            "poseT": poseT[i * N_PER_CORE:(i + 1) * N_PER_CORE],
            "pbs4": pbs4[i * (N_PER_CORE // 4):(i + 1) * (N_PER_CORE // 4)],
            "waT": waT, "wb4": wb4,
            "ct": ct, "bias4": bias4,
        })
    res = run_bass_kernel_spmd(nc, in_maps, list(range(N_CORES)), **spmd_kwargs)
    outs = [res.results[i]["out"] for i in range(N_CORES)]
    full = np.concatenate(outs, axis=0)          # [N//4, T//512, 128, 512]
    full = full.reshape(N // 4, T // 512, 4, 32, 512)[:, :, :, 0:F_OUT, :]
    # -> [N//4, 4, T//512, 512, 27] -> [N, T, 27]; cast before reshape so the
    # transpose materializes once, directly in f32
    full = full.transpose(0, 2, 1, 4, 3).astype(np.float32).reshape(N, T, F_OUT)
    return full, res


def kernel(**inputs) -> np.ndarray:
    out, _ = _run(inputs)
    return out



# revision 2
# speedup vs baseline: 1.8824x; 1.8824x over previous
import sys
import numpy as np

sys.path.insert(0, "/opt/trn_rl_repo")

import concourse.bass as bass  # noqa: E402
import concourse.bacc as bacc  # noqa: E402
import concourse.tile as tile  # noqa: E402
from concourse import mybir  # noqa: E402
from concourse.bass_utils import run_bass_kernel_spmd  # noqa: E402

import ml_dtypes  # noqa: E402

# Problem dims (hardcoded per spec)
N, T, V, C_IN, C_OUT, K, KT = 256, 2048, 9, 16, 3, 5, 9
F_IN = V * C_IN    # 144
F_OUT = V * C_OUT  # 27
N_CORES = 8
NPC = N // N_CORES  # 32 samples per core

F32 = mybir.dt.float32
F16 = mybir.dt.float16
F8 = mybir.dt.float8e3          # e3m4: 4 mantissa bits
NP_F8 = ml_dtypes.float8_e3m4

BLK = 120                       # t_out per conv block (halo 8 -> t_in 128)
NBLK = 18                       # 17 full + 1 partial (8 wide)
BPP = 6                         # blocks per panel
NPAN = 3                        # panels
PANW = BPP * BLK + 8            # 728 t_in cols per panel
TP = 4 + T + 120                # padded t cols in DRAM (head 4, tail 120)
SAMW = PANW * NPAN              # 2184 panel cols per sample (w/ overlaps)
NCHUNK = 4                      # sample-chunks per panel load
SPC = NPC // NCHUNK             # 8 samples per load chunk

_PROGRAM_CACHE = {}


def _build_program():
    nc = bacc.Bacc()

    xa = nc.declare_dram_parameter("xa", [NPC, 128, TP], F8, isOutput=False)
    xb = nc.declare_dram_parameter("xb", [NPC, 17, TP], F8, isOutput=False)
    w1 = nc.declare_dram_parameter("w1", [128, F_OUT], F16, isOutput=False)
    w2 = nc.declare_dram_parameter("w2", [17, F_OUT], F16, isOutput=False)
    bmat = nc.declare_dram_parameter("bmat", [128, 9 * BLK], F16, isOutput=False)
    btcn = nc.declare_dram_parameter("btcn", [BLK, C_OUT], F32, isOutput=False)
    # out[j, t', 288*o + 9*s + w]
    out = nc.declare_dram_parameter("out", [NBLK, BLK, NPC * F_OUT], F16,
                                    isOutput=True)

    with tile.TileContext(nc) as tc:
        with (
            tc.tile_pool(name="const", bufs=1) as cpool,
            tc.tile_pool(name="pa", bufs=2) as papool,
            tc.tile_pool(name="pb", bufs=2) as pbpool,
            tc.tile_pool(name="zt", bufs=2) as ztpool,
            tc.tile_pool(name="osb", bufs=3) as opool,
            tc.tile_pool(name="psza", bufs=2, space=bass.MemorySpace.PSUM) as psza_p,
            tc.tile_pool(name="pszb", bufs=2, space=bass.MemorySpace.PSUM) as pszb_p,
            tc.tile_pool(name="pso", bufs=1, space=bass.MemorySpace.PSUM) as pso_p,
        ):
            w1_sb = cpool.tile([128, F_OUT], F16, tag="w1")
            w2_sb = cpool.tile([17, F_OUT], F16, tag="w2")
            bmat_sb = cpool.tile([128, 9 * BLK], F16, tag="bmat")
            btcn_sb = cpool.tile([BLK, C_OUT], F32, tag="btcn")
            nc.sync.dma_start(w1_sb[:], w1[:])
            nc.sync.dma_start(w2_sb[:], w2[:])
            nc.sync.dma_start(bmat_sb[:], bmat[:])
            nc.sync.dma_start(btcn_sb[:], btcn[:])

            for p in range(NPAN):
                pa_t = papool.tile([128, NPC * PANW], F8, tag="pa")
                pb_t = pbpool.tile([17, NPC * PANW], F8, tag="pb")
                pa_v = pa_t[:].rearrange("p (s c) -> p s c", s=NPC)
                pb_v = pb_t[:].rearrange("p (s c) -> p s c", s=NPC)
                c0 = 720 * p
                for ch in range(NCHUNK):
                    s0 = ch * SPC
                    nc.sync.dma_start(
                        pa_v[:, s0:s0 + SPC, :],
                        xa[s0:s0 + SPC, :, c0:c0 + PANW].transpose([1, 0, 2]),
                    )
                    nc.sync.dma_start(
                        pb_v[:, s0:s0 + SPC, :],
                        xb[s0:s0 + SPC, :, c0:c0 + PANW].transpose([1, 0, 2]),
                    )

                for b in range(BPP):
                    j = BPP * p + b
                    wout = T - BLK * (NBLK - 1) if j == NBLK - 1 else BLK
                    zt_t = ztpool.tile([128, NPC * F_OUT], F16, tag="zt")
                    for half in range(2):
                        psz = (psza_p if half == 0 else pszb_p).tile(
                            [128, 16 * F_OUT], F32, tag=f"psz{half}")
                        for s16 in range(16):
                            s = 16 * half + s16
                            col0 = PANW * s + BLK * b
                            oc = F_OUT * s16
                            nc.tensor.matmul(
                                psz[:, oc:oc + F_OUT],
                                pa_t[:, col0:col0 + 128], w1_sb[:],
                                start=(s16 == 0), stop=False,
                            )
                            nc.tensor.matmul(
                                psz[:, oc:oc + F_OUT],
                                pb_t[:, col0:col0 + 128], w2_sb[:],
                                start=False, stop=(s16 == 15),
                            )
                        nc.vector.tensor_copy(
                            zt_t[:, 432 * half:432 * half + 432], psz[:])

                    osb = opool.tile([BLK, NPC * F_OUT], F16, tag="osb")
                    zt_v = zt_t[:].rearrange("p (s w i) -> p s w i", w=V, i=C_OUT)
                    for o in range(C_OUT):
                        pso = pso_p.tile([BLK, NPC * V], F32, tag=f"pso{o}",
                                         name=f"pso{o}")
                        for i in range(C_OUT):
                            kcol = BLK * (3 * i + o)
                            nc.tensor.matmul(
                                pso[0:wout, :],
                                bmat_sb[:, kcol:kcol + wout],
                                zt_v[:, :, :, i],
                                start=(i == 0), stop=(i == 2),
                            )
                        nc.scalar.activation(
                            osb[0:wout, 288 * o:288 * o + 288], pso[0:wout, :],
                            mybir.ActivationFunctionType.Lrelu,
                            bias=btcn_sb[0:wout, o:o + 1], alpha=0.01,
                        )
                    nc.scalar.dma_start(out[j, 0:wout, :], osb[0:wout, :])

    nc.finalize()
    return nc


def _host_consts(A, W_gcn, b_gcn, W_tcn, b_tcn):
    A = np.asarray(A, np.float32)
    W_gcn = np.asarray(W_gcn, np.float32)
    b_gcn = np.asarray(b_gcn, np.float32)
    W_tcn = np.asarray(W_tcn, np.float32)
    b_tcn = np.asarray(b_tcn, np.float32)

    # W_eff[(v,c),(w,o)] = sum_k W_gcn[k,o,c] A[k,v,w]; z = x^T W_eff + b_eff
    W_eff = np.einsum("koc,kvw->vcwo", W_gcn, A).reshape(F_IN, F_OUT)
    b_eff = np.einsum("ko,kw->wo", b_gcn, A.sum(axis=1)).reshape(F_OUT)
    w1 = W_eff[:128].astype(np.float16)
    w2 = np.vstack([W_eff[128:], b_eff[None]]).astype(np.float16)

    # banded conv matrices: bmat[:, 120*(3i+o)+c][r] = W_tcn[o,i,8-(r-c)]
    bmat = np.zeros((128, 9 * BLK), np.float32)
    r = np.arange(128)[:, None]
    c = np.arange(BLK)[None, :]
    d = r - c
    mask = (d >= 0) & (d <= 8)
    dd = np.clip(d, 0, 8)
    for i in range(3):
        for o in range(3):
            blk = np.where(mask, W_tcn[o, i, 8 - dd, 0], 0.0)
            bmat[:, BLK * (3 * i + o):BLK * (3 * i + o + 1)] = blk
    bmat = bmat.astype(np.float16)

    btcn = np.tile(b_tcn[None, :], (BLK, 1)).astype(np.float32)
    return w1, w2, bmat, btcn


def _host_inputs(pose):
    # channel-major, fp8, padded cols: col u <-> t = u - 4
    x = np.ascontiguousarray(pose.transpose(0, 2, 1))  # [N, 144, T] f32
    xa = np.zeros((N, 128, TP), NP_F8)
    xb = np.zeros((N, 17, TP), NP_F8)
    xa[:, :, 4:4 + T] = x[:, :128].astype(NP_F8)
    xb[:, :16, 4:4 + T] = x[:, 128:].astype(NP_F8)
    xb[:, 16, 4:4 + T] = NP_F8(1.0)
    return xa, xb


def _run(inputs, **spmd_kwargs):
    pose = np.asarray(inputs["pose_feats"], np.float32)
    xa, xb = _host_inputs(pose)
    w1, w2, bmat, btcn = _host_consts(
        inputs["A"], inputs["W_gcn"], inputs["b_gcn"],
        inputs["W_tcn"], inputs["b_tcn"])

    if "prog" not in _PROGRAM_CACHE:
        _PROGRAM_CACHE["prog"] = _build_program()
    nc = _PROGRAM_CACHE["prog"]

    in_maps = []
    for i in range(N_CORES):
        sl = slice(i * NPC, (i + 1) * NPC)
        in_maps.append({
            "xa": xa[sl], "xb": xb[sl],
            "w1": w1, "w2": w2, "bmat": bmat, "btcn": btcn,
        })
    res = run_bass_kernel_spmd(nc, in_maps, list(range(N_CORES)), **spmd_kwargs)
    outs = [res.results[i]["out"] for i in range(N_CORES)]
    full = np.stack(outs, axis=0)              # [8, 18, 120, 864]
    # col = 288*o + 9*s + w ; y[core*32+s, 120*j+t', 3*w+o]
    full = full.reshape(N_CORES, NBLK, BLK, C_OUT, NPC, V).astype(np.float32)
    full = full.transpose(0, 4, 1, 2, 5, 3)    # [core, s, j, t', w, o]
    full = full.reshape(N, NBLK * BLK, F_OUT)[:, :T, :]
    return np.ascontiguousarray(full), res


def kernel(**inputs) -> np.ndarray:
    out, _ = _run(inputs)
    return out


# revision 5
# speedup vs baseline: 1.9323x; 1.0265x over previous
import sys
import numpy as np

sys.path.insert(0, "/opt/trn_rl_repo")

import concourse.bass as bass  # noqa: E402
import concourse.bacc as bacc  # noqa: E402
import concourse.tile as tile  # noqa: E402
from concourse import mybir  # noqa: E402
from concourse.bass_utils import run_bass_kernel_spmd  # noqa: E402

import ml_dtypes  # noqa: E402

# Problem dims (hardcoded per spec)
N, T, V, C_IN, C_OUT, K, KT = 256, 2048, 9, 16, 3, 5, 9
F_IN = V * C_IN    # 144
F_OUT = V * C_OUT  # 27
N_CORES = 8
NPC = N // N_CORES  # 32 samples per core

F32 = mybir.dt.float32
F16 = mybir.dt.float16
F8 = mybir.dt.float8e3          # e3m4: 4 mantissa bits
NP_F8 = ml_dtypes.float8_e3m4

BLK = 120                       # t_out per conv block (halo 8 -> t_in 128)
NBLK = 18                       # 17 full + 1 partial (8 wide)
BPP = 6                         # blocks per panel
NPAN = 3                        # panels
PANW = BPP * BLK + 8            # 728 t_in cols per panel
TP = 4 + T + 120                # padded t cols in DRAM (head 4, tail 120)
SAMW = PANW * NPAN              # 2184 panel cols per sample (w/ overlaps)
NCHUNK = 4                      # sample-chunks per panel load
SPC = NPC // NCHUNK             # 8 samples per load chunk

_PROGRAM_CACHE = {}


def _build_program():
    nc = bacc.Bacc()

    xa = nc.declare_dram_parameter("xa", [NPC, 128, TP], F8, isOutput=False)
    xb = nc.declare_dram_parameter("xb", [NPC, 17, TP], F8, isOutput=False)
    w1 = nc.declare_dram_parameter("w1", [128, F_OUT], F16, isOutput=False)
    w2 = nc.declare_dram_parameter("w2", [17, F_OUT], F16, isOutput=False)
    bmat = nc.declare_dram_parameter("bmat", [128, 9 * BLK], F16, isOutput=False)
    btcn = nc.declare_dram_parameter("btcn", [BLK, C_OUT], F32, isOutput=False)
    # out[j, t', 288*o + 9*s + w]
    out = nc.declare_dram_parameter("out", [NBLK, BLK, NPC * F_OUT], F16,
                                    isOutput=True)

    with tile.TileContext(nc) as tc:
        with (
            tc.tile_pool(name="const", bufs=1) as cpool,
            tc.tile_pool(name="pa", bufs=3) as papool,
            tc.tile_pool(name="pb", bufs=3) as pbpool,
            tc.tile_pool(name="zt", bufs=2) as ztpool,
            tc.tile_pool(name="osb", bufs=3) as opool,
            tc.tile_pool(name="psza", bufs=2, space=bass.MemorySpace.PSUM) as psza_p,
            tc.tile_pool(name="pszb", bufs=2, space=bass.MemorySpace.PSUM) as pszb_p,
            tc.tile_pool(name="pso", bufs=1, space=bass.MemorySpace.PSUM) as pso_p,
        ):
            w1_sb = cpool.tile([128, F_OUT], F16, tag="w1")
            w2_sb = cpool.tile([17, F_OUT], F16, tag="w2")
            bmat_sb = cpool.tile([128, 9 * BLK], F16, tag="bmat")
            btcn_sb = cpool.tile([BLK, C_OUT], F32, tag="btcn")
            nc.gpsimd.dma_start(w1_sb[:], w1[:])
            nc.gpsimd.dma_start(w2_sb[:], w2[:])
            nc.gpsimd.dma_start(bmat_sb[:], bmat[:])
            nc.gpsimd.dma_start(btcn_sb[:], btcn[:])

            for p in range(NPAN):
                pa_t = papool.tile([128, NPC * PANW], F8, tag="pa")
                pb_t = pbpool.tile([17, NPC * PANW], F8, tag="pb")
                pa_v = pa_t[:].rearrange("p (s c) -> p s c", s=NPC)
                pb_v = pb_t[:].rearrange("p (s c) -> p s c", s=NPC)
                c0 = 720 * p
                for ch in range(NCHUNK):
                    s0 = ch * SPC
                    nc.sync.dma_start(
                        pa_v[:, s0:s0 + SPC, :],
                        xa[s0:s0 + SPC, :, c0:c0 + PANW].transpose([1, 0, 2]),
                    )
                    nc.sync.dma_start(
                        pb_v[:, s0:s0 + SPC, :],
                        xb[s0:s0 + SPC, :, c0:c0 + PANW].transpose([1, 0, 2]),
                    )

                for b in range(BPP):
                    j = BPP * p + b
                    wout = T - BLK * (NBLK - 1) if j == NBLK - 1 else BLK
                    zt_t = ztpool.tile([128, NPC * F_OUT], F16, tag="zt")
                    for half in range(2):
                        psz = (psza_p if half == 0 else pszb_p).tile(
                            [128, 16 * F_OUT], F32, tag=f"psz{half}")
                        for s16 in range(16):
                            s = 16 * half + s16
                            col0 = PANW * s + BLK * b
                            oc = F_OUT * s16
                            nc.tensor.matmul(
                                psz[:, oc:oc + F_OUT],
                                pa_t[:, col0:col0 + 128], w1_sb[:],
                                start=(s16 == 0), stop=False,
                            )
                            nc.tensor.matmul(
                                psz[:, oc:oc + F_OUT],
                                pb_t[:, col0:col0 + 128], w2_sb[:],
                                start=False, stop=(s16 == 15),
                            )
                        nc.vector.tensor_copy(
                            zt_t[:, 432 * half:432 * half + 432], psz[:])

                    osb = opool.tile([BLK, NPC * F_OUT], F16, tag="osb")
                    zt_v = zt_t[:].rearrange("p (s w i) -> p s w i", w=V, i=C_OUT)
                    for o in range(C_OUT):
                        pso = pso_p.tile([BLK, NPC * V], F32, tag=f"pso{o}",
                                         name=f"pso{o}")
                        for i in range(C_OUT):
                            kcol = BLK * (3 * i + o)
                            nc.tensor.matmul(
                                pso[0:wout, :],
                                bmat_sb[:, kcol:kcol + wout],
                                zt_v[:, :, :, i],
                                start=(i == 0), stop=(i == 2),
                            )
                        nc.scalar.activation(
                            osb[0:wout, 288 * o:288 * o + 288], pso[0:wout, :],
                            mybir.ActivationFunctionType.Lrelu,
                            bias=btcn_sb[0:wout, o:o + 1], alpha=0.01,
                        )
                    nc.scalar.dma_start(out[j, 0:wout, :], osb[0:wout, :])

    nc.finalize()
    return nc


def _host_consts(A, W_gcn, b_gcn, W_tcn, b_tcn):
    A = np.asarray(A, np.float32)
    W_gcn = np.asarray(W_gcn, np.float32)
    b_gcn = np.asarray(b_gcn, np.float32)
    W_tcn = np.asarray(W_tcn, np.float32)
    b_tcn = np.asarray(b_tcn, np.float32)

    # W_eff[(v,c),(w,o)] = sum_k W_gcn[k,o,c] A[k,v,w]; z = x^T W_eff + b_eff
    W_eff = np.einsum("koc,kvw->vcwo", W_gcn, A).reshape(F_IN, F_OUT)
    b_eff = np.einsum("ko,kw->wo", b_gcn, A.sum(axis=1)).reshape(F_OUT)
    w1 = W_eff[:128].astype(np.float16)
    w2 = np.vstack([W_eff[128:], b_eff[None]]).astype(np.float16)

    # banded conv matrices: bmat[:, 120*(3i+o)+c][r] = W_tcn[o,i,8-(r-c)]
    bmat = np.zeros((128, 9 * BLK), np.float32)
    r = np.arange(128)[:, None]
    c = np.arange(BLK)[None, :]
    d = r - c
    mask = (d >= 0) & (d <= 8)
    dd = np.clip(d, 0, 8)
    for i in range(3):
        for o in range(3):
            blk = np.where(mask, W_tcn[o, i, 8 - dd, 0], 0.0)
            bmat[:, BLK * (3 * i + o):BLK * (3 * i + o + 1)] = blk
    bmat = bmat.astype(np.float16)

    btcn = np.tile(b_tcn[None, :], (BLK, 1)).astype(np.float32)
    return w1, w2, bmat, btcn


def _host_inputs(pose):
    # channel-major, fp8, padded cols: col u <-> t = u - 4
    x = np.ascontiguousarray(pose.transpose(0, 2, 1))  # [N, 144, T] f32
    xa = np.zeros((N, 128, TP), NP_F8)
    xb = np.zeros((N, 17, TP), NP_F8)
    xa[:, :, 4:4 + T] = x[:, :128].astype(NP_F8)
    xb[:, :16, 4:4 + T] = x[:, 128:].astype(NP_F8)
    xb[:, 16, 4:4 + T] = NP_F8(1.0)
    return xa, xb


def _run(inputs, **spmd_kwargs):
    pose = np.asarray(inputs["pose_feats"], np.float32)
    xa, xb = _host_inputs(pose)
    w1, w2, bmat, btcn = _host_consts(
        inputs["A"], inputs["W_gcn"], inputs["b_gcn"],
        inputs["W_tcn"], inputs["b_tcn"])

    if "prog" not in _PROGRAM_CACHE:
        _PROGRAM_CACHE["prog"] = _build_program()
    nc = _PROGRAM_CACHE["prog"]

    in_maps = []
    for i in range(N_CORES):
        sl = slice(i * NPC, (i + 1) * NPC)
        in_maps.append({
            "xa": xa[sl], "xb": xb[sl],
            "w1": w1, "w2": w2, "bmat": bmat, "btcn": btcn,
        })
    res = run_bass_kernel_spmd(nc, in_maps, list(range(N_CORES)), **spmd_kwargs)
    outs = [res.results[i]["out"] for i in range(N_CORES)]
    full = np.stack(outs, axis=0)              # [8, 18, 120, 864]
    # col = 288*o + 9*s + w ; y[core*32+s, 120*j+t', 3*w+o]
    full = full.reshape(N_CORES, NBLK, BLK, C_OUT, NPC, V).astype(np.float32)
    full = full.transpose(0, 4, 1, 2, 5, 3)    # [core, s, j, t', w, o]
    full = full.reshape(N, NBLK * BLK, F_OUT)[:, :T, :]
    return np.ascontiguousarray(full), res


def kernel(**inputs) -> np.ndarray:
    out, _ = _run(inputs)
    return out
